# revision 5
# baseline (speedup 1.0000x reference)
"""2-layer GAT (graph attention) on 8 Trainium2 NeuronCores.

Sharding: query/node rows split 8 ways (512 rows per core). Attention scores are
computed transposed ([key_block=128, q=512]) so the probability tiles feed the
TensorE directly as lhsT with no on-chip transpose of the attention matrix.
Score vectors come from fused weights (e_src = x @ (W1 @ a_src)), leaky-relu and
the bias add ride the ScalarE activation (Prelu with per-partition bias), the
edge mask is applied multiplicatively after exp (identical to -inf masking), and
softmax denominators come free via a ones-column appended to the value matrix.
Layer 2 needs full layer-1 node features; each core computes its 512 rows and a
small packed [512, 18] payload (h2 | ones | e2_dst) is AllGathered.
"""

import numpy as np
import ml_dtypes

import concourse.bass as bass
import concourse.tile as tile
from concourse import bacc, mybir
from concourse.bass_utils import run_bass_kernel_spmd
from concourse.masks import make_identity

P = 128
N, F, O, H, C = 4096, 512, 256, 4, 16
NCORES = 8
Q = N // NCORES          # 512 query rows per core
QC = Q // P              # 4 query chunks
MB = N // P              # 32 key blocks
KB = F // P              # 4 contraction blocks over F
OB = O // P              # 2 contraction blocks over O
ALPHA = 0.2
PAY = C + 2              # payload cols: 0:16 h2, 16 ones, 17 e2_dst

bf16 = mybir.dt.bfloat16
f32 = mybir.dt.float32
AF = mybir.ActivationFunctionType
ALU = mybir.AluOpType
AX = mybir.AxisListType


def _build(reps=1):
    nc = bacc.Bacc("TRN2", target_bir_lowering=False, debug=False,
                   num_devices=NCORES)

    xT_d = nc.dram_tensor("xT", [F, N], bf16, kind="ExternalInput").ap()
    xTq_d = nc.dram_tensor("xTq", [F, Q], bf16, kind="ExternalInput").ap()
    maskT_d = nc.dram_tensor("maskT", [N, Q], bf16, kind="ExternalInput").ap()
    w1_d = nc.dram_tensor("w1", [F, H * O], bf16, kind="ExternalInput").ap()
    wsd_d = nc.dram_tensor("wsd", [F, 2 * H], bf16, kind="ExternalInput").ap()
    w2p_d = nc.dram_tensor("w2p", [O, PAY], bf16, kind="ExternalInput").ap()
    out_d = nc.dram_tensor("out", [Q, C], f32, kind="ExternalOutput").ap()

    with tile.TileContext(nc) as tc:
        for _ in range(reps):
            _emit(tc, xT_d, xTq_d, maskT_d, w1_d, wsd_d, w2p_d, out_d)
    nc.compile()
    return nc


def _emit(tc, xT_d, xTq_d, maskT_d, w1_d, wsd_d, w2p_d, out_d):
    nc = tc.nc
    with tc.tile_pool(name="singles", bufs=1) as singles:
        # ---- persistent SBUF tensors ----
        xT_sb = singles.tile([P, KB, N], bf16)
        xTq_sb = singles.tile([P, KB, Q], bf16)
        maskT_sb = singles.tile([P, MB, Q], bf16)
        w1_sb = singles.tile([P, KB, H * O], bf16)
        wsd_sb = singles.tile([P, KB, 2 * H], bf16)
        w2p_sb = singles.tile([P, OB, PAY], bf16)
        ones1 = singles.tile([1, P], f32)
        ident = singles.tile([P, P], bf16)
        h_sb = singles.tile([P, MB, H, O + 1], bf16)
        edst_sb = singles.tile([P, MB, 2 * H], f32)
        esrcb = singles.tile([P, H, Q], f32)
        x2acc = singles.tile([P, QC, O], f32)
        x2bf = singles.tile([P, QC, O], bf16)
        x2T = singles.tile([P, OB, Q], bf16)
        pay_sb = singles.tile([P, QC, PAY], bf16)
        h2g_sb = singles.tile([P, MB, PAY], bf16)
        e2b = singles.tile([P, Q], f32)

        # ---- input DMAs (split for queue parallelism) ----
        xT_r = xT_d.rearrange("(kb p) n -> p kb n", p=P)
        for kb in range(KB):
            for half in range(2):
                s = bass.ds(half * (N // 2), N // 2)
                nc.sync.dma_start(xT_sb[:, kb, s], xT_r[:, kb, s])
        nc.sync.dma_start(xTq_sb[:], xTq_d.rearrange("(kb p) q -> p kb q", p=P))
        maskT_r = maskT_d.rearrange("(b p) q -> p b q", p=P)
        for g in range(8):
            s = bass.ts(g, MB // 8)
            nc.sync.dma_start(maskT_sb[:, s, :], maskT_r[:, s, :])
        w1_r = w1_d.rearrange("(kb p) c -> p kb c", p=P)
        for half in range(2):
            s = bass.ds(half * (H * O // 2), H * O // 2)
            nc.sync.dma_start(w1_sb[:, :, s], w1_r[:, :, s])
        nc.sync.dma_start(wsd_sb[:], wsd_d.rearrange("(kb p) c -> p kb c", p=P))
        nc.sync.dma_start(w2p_sb[:], w2p_d.rearrange("(ob p) c -> p ob c", p=P))

        nc.vector.memset(ones1[:], 1.0)
        make_identity(nc, ident[:])
        nc.vector.memset(h_sb[:, :, :, O], 1.0)

        # ---- phase B: score vectors ----
        with tc.tile_pool(name="vec_psum", bufs=2, space="PSUM") as vpp, \
             tc.tile_pool(name="vec_sb", bufs=2) as vsb:
            for h in range(H):
                ps_es = vpp.tile([1, Q], f32, tag="ps_es")
                for kb in range(KB):
                    nc.tensor.matmul(ps_es[:], wsd_sb[:, kb, h:h + 1],
                                     xTq_sb[:, kb, :],
                                     start=(kb == 0), stop=(kb == KB - 1))
                esrcT = vsb.tile([1, Q], f32, tag="esrcT")
                nc.vector.tensor_copy(esrcT[:], ps_es[:])
                ps_b = vpp.tile([P, Q], f32, tag="ps_b")
                nc.tensor.matmul(ps_b[:], ones1[:], esrcT[:],
                                 start=True, stop=True)
                nc.vector.tensor_copy(esrcb[:, h, :], ps_b[:])
            for mb in range(MB):
                ps_E = vpp.tile([P, 2 * H], f32, tag="ps_E")
                for kb in range(KB):
                    nc.tensor.matmul(ps_E[:], xT_sb[:, kb, bass.ts(mb, P)],
                                     wsd_sb[:, kb, :],
                                     start=(kb == 0), stop=(kb == KB - 1))
                nc.vector.tensor_copy(edst_sb[:, mb, :], ps_E[:])

        # ---- phase C: h = x @ W1 for all nodes (replicated) ----
        with tc.tile_pool(name="h_psum", bufs=2, space="PSUM") as hpp:
            for mb in range(MB):
                ps_h = hpp.tile([P, H * O], f32, tag="ps_h")
                for half in range(2):
                    s = bass.ds(half * 512, 512)
                    for kb in range(KB):
                        nc.tensor.matmul(ps_h[:, s], xT_sb[:, kb, bass.ts(mb, P)],
                                         w1_sb[:, kb, s],
                                         start=(kb == 0), stop=(kb == KB - 1))
                for h in range(H):
                    nc.vector.tensor_copy(h_sb[:, mb, h, 0:O],
                                          ps_h[:, bass.ts(h, O)])

        # ---- phase D: layer-1 attention, head-outer ----
        with tc.tile_pool(name="acc_psum", bufs=2, space="PSUM") as accp, \
             tc.tile_pool(name="p_pool", bufs=4) as pp, \
             tc.tile_pool(name="small1", bufs=4) as sp1:
            for h in range(H):
                accs = []
                for qc in range(QC):
                    accs.append(accp.tile([P, O + 1], f32, tag=f"acc{qc}", name=f"acc{qc}"))
                for mb in range(MB):
                    zt = pp.tile([P, Q], f32, tag="zt")
                    nc.scalar.activation(zt[:], esrcb[:, h, :], AF.Prelu,
                                         bias=edst_sb[:, mb, H + h:H + h + 1],
                                         scale=1.0,
                                         alpha=ALPHA)
                    pt = pp.tile([P, Q], bf16, tag="pt")
                    nc.scalar.activation(pt[:], zt[:], AF.Exp)
                    pm = pp.tile([P, Q], bf16, tag="pm")
                    nc.vector.tensor_mul(pm[:], pt[:], maskT_sb[:, mb, :])
                    for qc in range(QC):
                        nc.tensor.matmul(accs[qc][:], pm[:, bass.ts(qc, P)],
                                         h_sb[:, mb, h, :],
                                         start=(mb == 0), stop=(mb == MB - 1))
                for qc in range(QC):
                    r = sp1.tile([P, 1], f32, tag="r")
                    nc.vector.reciprocal(r[:], accs[qc][:, O:O + 1])
                    nc.vector.tensor_scalar_mul(r[:], r[:], 1.0 / H)
                    if h == 0:
                        nc.vector.tensor_scalar_mul(x2acc[:, qc, :],
                                                    accs[qc][:, 0:O], r[:])
                    else:
                        nc.vector.scalar_tensor_tensor(
                            x2acc[:, qc, :], accs[qc][:, 0:O], r[:],
                            x2acc[:, qc, :], op0=ALU.mult, op1=ALU.add)

        # ---- phase E: relu, transpose, layer-2 projections ----
        nc.vector.tensor_relu(x2bf[:], x2acc[:])
        with tc.tile_pool(name="l2_psum", bufs=2, space="PSUM") as lpp, \
             tc.tile_pool(name="l2_sb", bufs=2) as lsb:
            for qc in range(QC):
                for ob in range(OB):
                    tp = lpp.tile([P, P], bf16, tag="tp")
                    nc.tensor.transpose(tp[:], x2bf[:, qc, bass.ts(ob, P)],
                                        ident[:])
                    nc.vector.tensor_copy(x2T[:, ob, bass.ts(qc, P)], tp[:])
            for qc in range(QC):
                ps2 = lpp.tile([P, C + 1], f32, tag="ps2")
                for ob in range(OB):
                    nc.tensor.matmul(ps2[:], x2T[:, ob, bass.ts(qc, P)],
                                     w2p_sb[:, ob, 0:C + 1],
                                     start=(ob == 0), stop=(ob == OB - 1))
                nc.vector.tensor_copy(pay_sb[:, qc, 0:C], ps2[:, 0:C])
                nc.vector.tensor_copy(pay_sb[:, qc, C + 1:C + 2],
                                      ps2[:, C:C + 1])
            nc.vector.memset(pay_sb[:, :, C], 1.0)
            ps_e2 = lpp.tile([1, Q], f32, tag="ps_e2")
            for ob in range(OB):
                nc.tensor.matmul(ps_e2[:], w2p_sb[:, ob, C + 1:C + 2],
                                 x2T[:, ob, :],
                                 start=(ob == 0), stop=(ob == OB - 1))
            e2srcT = lsb.tile([1, Q], f32, tag="e2srcT")
            nc.vector.tensor_copy(e2srcT[:], ps_e2[:])
            ps_b2 = lpp.tile([P, Q], f32, tag="ps_b2")
            nc.tensor.matmul(ps_b2[:], ones1[:], e2srcT[:],
                             start=True, stop=True)
            nc.vector.tensor_copy(e2b[:], ps_b2[:])

        # ---- phase F: AllGather packed payload ----
        with tc.tile_pool(name="dram", bufs=1, space="DRAM") as dram:
            pay_d = dram.tile([Q, PAY], bf16)
            gath_d = dram.tile([N, PAY], bf16)
            nc.sync.dma_start(pay_d.rearrange("(qc p) c -> p qc c", p=P),
                              pay_sb[:])
            nc.gpsimd.collective_compute(
                "AllGather", ALU.bypass,
                replica_groups=[list(range(NCORES))],
                ins=[pay_d.opt()], outs=[gath_d.opt()])
            nc.sync.dma_start(h2g_sb[:],
                              gath_d.rearrange("(b p) c -> p b c", p=P))

            # ---- phase G: layer-2 attention ----
            with tc.tile_pool(name="acc2_psum", bufs=2, space="PSUM") as acc2p, \
                 tc.tile_pool(name="p2_pool", bufs=4) as pp2, \
                 tc.tile_pool(name="small2", bufs=4) as sp2:
                accs2 = []
                for qc in range(QC):
                    accs2.append(acc2p.tile([P, C + 1], f32, tag=f"a2_{qc}", name=f"a2_{qc}"))
                for mb in range(MB):
                    bcol = sp2.tile([P, 1], f32, tag="bcol")
                    nc.vector.tensor_copy(bcol[:], h2g_sb[:, mb, C + 1:C + 2])
                    z2 = pp2.tile([P, Q], f32, tag="z2")
                    nc.scalar.activation(z2[:], e2b[:], AF.Prelu,
                                         bias=bcol[:], scale=1.0, alpha=ALPHA)
                    p2 = pp2.tile([P, Q], bf16, tag="p2")
                    nc.scalar.activation(p2[:], z2[:], AF.Exp)
                    pm2 = pp2.tile([P, Q], bf16, tag="pm2")
                    nc.vector.tensor_mul(pm2[:], p2[:], maskT_sb[:, mb, :])
                    for qc in range(QC):
                        nc.tensor.matmul(accs2[qc][:], pm2[:, bass.ts(qc, P)],
                                         h2g_sb[:, mb, 0:C + 1],
                                         start=(mb == 0), stop=(mb == MB - 1))
                for qc in range(QC):
                    r2 = sp2.tile([P, 1], f32, tag="r2")
                    nc.vector.reciprocal(r2[:], accs2[qc][:, C:C + 1])
                    logits = sp2.tile([P, C], f32, tag="logits")
                    nc.vector.tensor_scalar_mul(logits[:], accs2[qc][:, 0:C],
                                                r2[:])
                    negmax = sp2.tile([P, 1], f32, tag="negmax")
                    nc.vector.reduce_max(negmax[:], logits[:], axis=AX.X,
                                         negate=True)
                    expt = sp2.tile([P, C], f32, tag="expt")
                    ssum = sp2.tile([P, 1], f32, tag="ssum")
                    nc.scalar.activation(expt[:], logits[:], AF.Exp,
                                         bias=negmax[:], accum_out=ssum[:])
                    lse = sp2.tile([P, 1], f32, tag="lse")
                    nc.scalar.activation(lse[:], ssum[:], AF.Ln)
                    res = sp2.tile([P, C], f32, tag="res")
                    nc.vector.tensor_scalar(res[:], logits[:], negmax[:],
                                            lse[:], ALU.add, ALU.subtract)
                    nc.sync.dma_start(out_d[bass.ts(qc, P), :], res[:])


_CACHED = None


def _get_nc():
    global _CACHED
    if _CACHED is None:
        _CACHED = _build()
    return _CACHED


def kernel(x, adj, W1, a1, W2, a2):
    x = np.asarray(x, dtype=np.float32)
    adj = np.asarray(adj)
    W1 = np.asarray(W1, dtype=np.float32)
    a1 = np.asarray(a1, dtype=np.float32)
    W2 = np.asarray(W2, dtype=np.float32)
    a2 = np.asarray(a2, dtype=np.float32)

    bf = ml_dtypes.bfloat16
    xT = np.ascontiguousarray(x.T).astype(bf)                     # [F, N]
    # fused score weights: e_src = x @ (W1 @ a_src), e_dst likewise
    wsrc = np.einsum("hfo,ho->fh", W1, a1[:, :O])                 # [F, H]
    wdst = np.einsum("hfo,ho->fh", W1, a1[:, O:])                 # [F, H]
    wsd = np.concatenate([wsrc, wdst], axis=1).astype(bf)         # [F, 2H]
    w1cat = np.concatenate([W1[h] for h in range(H)], 1).astype(bf)  # [F, H*O]
    w2p = np.zeros((O, PAY), np.float32)
    w2p[:, 0:C] = W2[0]
    w2p[:, C] = W2[0] @ a2[0, C:]      # e2_dst vector
    w2p[:, C + 1] = W2[0] @ a2[0, :C]  # e2_src vector
    w2p = w2p.astype(bf)

    adj_on = adj > 0
    in_maps = []
    for c in range(NCORES):
        rows = slice(c * Q, (c + 1) * Q)
        in_maps.append({
            "xT": xT,
            "xTq": np.ascontiguousarray(xT[:, rows]),
            "maskT": np.ascontiguousarray(adj_on[rows, :].T).astype(bf),
            "w1": w1cat,
            "wsd": wsd,
            "w2p": w2p,
        })

    nc = _get_nc()
    res = run_bass_kernel_spmd(nc, in_maps, core_ids=list(range(NCORES)))
    return np.concatenate([res.results[c]["out"] for c in range(NCORES)], 0)


# revision 10
# speedup vs baseline: 1.2893x; 1.2893x over previous
"""2-layer GAT (graph attention) on 8 Trainium2 NeuronCores.

Sharding: query/node rows split 8 ways (512 rows per core). Attention scores are
computed transposed ([key_block=128, q=512]) so the probability tiles feed the
TensorE directly as lhsT with no on-chip transpose of the attention matrix.
Score vectors come from fused weights (e_src = x @ (W1 @ a_src)), leaky-relu and
the bias add ride the ScalarE activation (Prelu with per-partition bias), the
edge mask is applied multiplicatively after exp (identical to -inf masking), and
softmax denominators come free via a ones-column appended to the value matrix.
Layer 2 needs full layer-1 node features; each core computes its 512 rows and a
small packed [512, 18] payload (h2 | ones | e2_dst) is AllGathered.
"""

import numpy as np
import ml_dtypes

import concourse.bass as bass
import concourse.tile as tile
from concourse import bacc, mybir
from concourse.bass_utils import run_bass_kernel_spmd
from concourse.masks import make_identity

P = 128
N, F, O, H, C = 4096, 512, 256, 4, 16
NCORES = 8
Q = N // NCORES          # 512 query rows per core
QC = Q // P              # 4 query chunks
MB = N // P              # 32 key blocks
KB = F // P              # 4 contraction blocks over F
OB = O // P              # 2 contraction blocks over O
ALPHA = 0.2
PAY = C + 2              # payload cols: 0:16 h2, 16 ones, 17 e2_dst
# leaky-relu engine routing, out of every 8 score tiles:
# first ACT_FRAC on ScalarE (Prelu), next DVE_FRAC on VectorE, rest on GpSimd
ACT_FRAC = 4
DVE_FRAC = 1

bf16 = mybir.dt.bfloat16
f32 = mybir.dt.float32
AF = mybir.ActivationFunctionType
ALU = mybir.AluOpType
AX = mybir.AxisListType


def _build(reps=1):
    nc = bacc.Bacc("TRN2", target_bir_lowering=False, debug=False,
                   num_devices=NCORES)

    xT_d = nc.dram_tensor("xT", [F, N], bf16, kind="ExternalInput").ap()
    xTq_d = nc.dram_tensor("xTq", [F, Q], bf16, kind="ExternalInput").ap()
    maskT_d = nc.dram_tensor("maskT", [N, Q], bf16, kind="ExternalInput").ap()
    w1_d = nc.dram_tensor("w1", [F, H * O], bf16, kind="ExternalInput").ap()
    wsd_d = nc.dram_tensor("wsd", [F, 2 * H], bf16, kind="ExternalInput").ap()
    w2p_d = nc.dram_tensor("w2p", [O, PAY], bf16, kind="ExternalInput").ap()
    out_d = nc.dram_tensor("out", [Q, C], f32, kind="ExternalOutput").ap()

    with tile.TileContext(nc) as tc:
        for _ in range(reps):
            _emit(tc, xT_d, xTq_d, maskT_d, w1_d, wsd_d, w2p_d, out_d)
    nc.compile()
    return nc


def _emit(tc, xT_d, xTq_d, maskT_d, w1_d, wsd_d, w2p_d, out_d):
    nc = tc.nc
    with tc.tile_pool(name="singles", bufs=1) as singles:
        # ---- persistent SBUF tensors ----
        xT_sb = singles.tile([P, KB, N], bf16)
        xTq_sb = singles.tile([P, KB, Q], bf16)
        maskT_sb = singles.tile([P, MB, Q], bf16)
        w1_sb = singles.tile([P, KB, H * O], bf16)
        wsd_sb = singles.tile([P, KB, 2 * H], bf16)
        w2p_sb = singles.tile([P, OB, PAY], bf16)
        ones1 = singles.tile([1, P], f32)
        ident = singles.tile([P, P], bf16)
        h_sb = singles.tile([P, MB, H, O + 1], bf16)
        edst_sb = singles.tile([P, MB, 2 * H], f32)
        esrcb = singles.tile([P, H, Q], f32)
        x2acc = singles.tile([P, QC, O], f32)
        x2bf = singles.tile([P, QC, O], bf16)
        x2T = singles.tile([P, OB, Q], bf16)
        pay_sb = singles.tile([P, QC, PAY], bf16)
        h2g_sb = singles.tile([P, MB, PAY], bf16)
        e2b = singles.tile([P, Q], f32)

        # ---- input DMAs (split for queue parallelism) ----
        xT_r = xT_d.rearrange("(kb p) n -> p kb n", p=P)
        for kb in range(KB):
            for half in range(2):
                s = bass.ds(half * (N // 2), N // 2)
                nc.sync.dma_start(xT_sb[:, kb, s], xT_r[:, kb, s])
        nc.sync.dma_start(xTq_sb[:], xTq_d.rearrange("(kb p) q -> p kb q", p=P))
        maskT_r = maskT_d.rearrange("(b p) q -> p b q", p=P)
        for g in range(8):
            s = bass.ts(g, MB // 8)
            nc.sync.dma_start(maskT_sb[:, s, :], maskT_r[:, s, :])
        w1_r = w1_d.rearrange("(kb p) c -> p kb c", p=P)
        for half in range(2):
            s = bass.ds(half * (H * O // 2), H * O // 2)
            nc.sync.dma_start(w1_sb[:, :, s], w1_r[:, :, s])
        nc.sync.dma_start(wsd_sb[:], wsd_d.rearrange("(kb p) c -> p kb c", p=P))
        nc.sync.dma_start(w2p_sb[:], w2p_d.rearrange("(ob p) c -> p ob c", p=P))

        nc.vector.memset(ones1[:], 1.0)
        make_identity(nc, ident[:])
        nc.vector.memset(h_sb[:, :, :, O], 1.0)

        # ---- phase B: score vectors ----
        with tc.tile_pool(name="vec_psum", bufs=2, space="PSUM") as vpp, \
             tc.tile_pool(name="vec_sb", bufs=2) as vsb:
            for h in range(H):
                ps_es = vpp.tile([1, Q], f32, tag="ps_es")
                for kb in range(KB):
                    nc.tensor.matmul(ps_es[:], wsd_sb[:, kb, h:h + 1],
                                     xTq_sb[:, kb, :],
                                     start=(kb == 0), stop=(kb == KB - 1))
                esrcT = vsb.tile([1, Q], f32, tag="esrcT")
                nc.vector.tensor_copy(esrcT[:], ps_es[:])
                ps_b = vpp.tile([P, Q], f32, tag="ps_b")
                nc.tensor.matmul(ps_b[:], ones1[:], esrcT[:],
                                 start=True, stop=True)
                nc.vector.tensor_copy(esrcb[:, h, :], ps_b[:])
            for mb in range(MB):
                ps_E = vpp.tile([P, 2 * H], f32, tag="ps_E")
                for kb in range(KB):
                    nc.tensor.matmul(ps_E[:], xT_sb[:, kb, bass.ts(mb, P)],
                                     wsd_sb[:, kb, :],
                                     start=(kb == 0), stop=(kb == KB - 1))
                nc.vector.tensor_copy(edst_sb[:, mb, :], ps_E[:])

        # ---- phase C: h = x @ W1 for all nodes (replicated) ----
        with tc.tile_pool(name="h_psum", bufs=2, space="PSUM") as hpp:
            for mb in range(MB):
                ps_h = hpp.tile([P, H * O], f32, tag="ps_h")
                for half in range(2):
                    s = bass.ds(half * 512, 512)
                    for kb in range(KB):
                        nc.tensor.matmul(ps_h[:, s], xT_sb[:, kb, bass.ts(mb, P)],
                                         w1_sb[:, kb, s],
                                         start=(kb == 0), stop=(kb == KB - 1))
                for h in range(H):
                    nc.vector.tensor_copy(h_sb[:, mb, h, 0:O],
                                          ps_h[:, bass.ts(h, O)])

        # ---- phase D: layer-1 attention, head-outer ----
        def leaky_exp(pool, esrc_ap, bias_ap, mask_ap, idx):
            """p = exp(leaky(esrc + bias)) * mask, with the leaky pass routed
            across engines by tile index to balance the attention phase."""
            route = idx % 8
            if route < ACT_FRAC:
                zt = pool.tile([P, Q], f32, tag="zt", name="zt")
                nc.scalar.activation(zt[:], esrc_ap, AF.Prelu, bias=bias_ap,
                                     scale=1.0, alpha=ALPHA)
            else:
                z0 = pool.tile([P, Q], f32, tag="z0", name="z0")
                nc.vector.tensor_scalar_add(z0[:], esrc_ap, bias_ap)
                zt = pool.tile([P, Q], f32, tag="zt", name="zt")
                nc.vector.scalar_tensor_tensor(zt[:], z0[:], ALPHA, z0[:],
                                               op0=ALU.mult, op1=ALU.max)
            pt = pool.tile([P, Q], bf16, tag="pt", name="pt")
            nc.scalar.activation(pt[:], zt[:], AF.Exp)
            pm = pool.tile([P, Q], bf16, tag="pm", name="pm")
            nc.vector.tensor_mul(pm[:], pt[:], mask_ap)
            return pm

        with tc.tile_pool(name="acc_psum", bufs=1, space="PSUM") as accp, \
             tc.tile_pool(name="p_pool", bufs=4) as pp, \
             tc.tile_pool(name="small1", bufs=4) as sp1:
            for h in range(H):
                accs = []
                for qc in range(QC):
                    accs.append(accp.tile([P, O + 1], f32, tag=f"acc{qc}", name=f"acc{qc}"))
                for mb in range(MB):
                    pm = leaky_exp(pp, esrcb[:, h, :],
                                   edst_sb[:, mb, H + h:H + h + 1],
                                   maskT_sb[:, mb, :], h * MB + mb)
                    for qc in range(QC):
                        nc.tensor.matmul(accs[qc][:], pm[:, bass.ts(qc, P)],
                                         h_sb[:, mb, h, :],
                                         start=(mb == 0), stop=(mb == MB - 1))
                for qc in range(QC):
                    r = sp1.tile([P, 1], f32, tag="r")
                    nc.vector.reciprocal(r[:], accs[qc][:, O:O + 1])
                    nc.vector.tensor_scalar_mul(r[:], r[:], 1.0 / H)
                    if h == 0:
                        nc.vector.tensor_scalar_mul(x2acc[:, qc, :],
                                                    accs[qc][:, 0:O], r[:])
                    else:
                        nc.vector.scalar_tensor_tensor(
                            x2acc[:, qc, :], accs[qc][:, 0:O], r[:],
                            x2acc[:, qc, :], op0=ALU.mult, op1=ALU.add)

        # ---- phase E: relu, transpose, layer-2 projections ----
        nc.vector.tensor_relu(x2bf[:], x2acc[:])
        with tc.tile_pool(name="l2_psum", bufs=2, space="PSUM") as lpp, \
             tc.tile_pool(name="l2_sb", bufs=2) as lsb:
            for qc in range(QC):
                for ob in range(OB):
                    tp = lpp.tile([P, P], bf16, tag="tp")
                    nc.tensor.transpose(tp[:], x2bf[:, qc, bass.ts(ob, P)],
                                        ident[:])
                    nc.vector.tensor_copy(x2T[:, ob, bass.ts(qc, P)], tp[:])
            for qc in range(QC):
                ps2 = lpp.tile([P, C + 1], f32, tag="ps2")
                for ob in range(OB):
                    nc.tensor.matmul(ps2[:], x2T[:, ob, bass.ts(qc, P)],
                                     w2p_sb[:, ob, 0:C + 1],
                                     start=(ob == 0), stop=(ob == OB - 1))
                nc.vector.tensor_copy(pay_sb[:, qc, 0:C], ps2[:, 0:C])
                nc.vector.tensor_copy(pay_sb[:, qc, C + 1:C + 2],
                                      ps2[:, C:C + 1])
            nc.vector.memset(pay_sb[:, :, C], 1.0)
            ps_e2 = lpp.tile([1, Q], f32, tag="ps_e2")
            for ob in range(OB):
                nc.tensor.matmul(ps_e2[:], w2p_sb[:, ob, C + 1:C + 2],
                                 x2T[:, ob, :],
                                 start=(ob == 0), stop=(ob == OB - 1))
            e2srcT = lsb.tile([1, Q], f32, tag="e2srcT")
            nc.vector.tensor_copy(e2srcT[:], ps_e2[:])
            ps_b2 = lpp.tile([P, Q], f32, tag="ps_b2")
            nc.tensor.matmul(ps_b2[:], ones1[:], e2srcT[:],
                             start=True, stop=True)
            nc.vector.tensor_copy(e2b[:], ps_b2[:])

        # ---- phase F: AllGather packed payload ----
        with tc.tile_pool(name="dram", bufs=1, space="DRAM") as dram:
            pay_d = dram.tile([Q, PAY], bf16)
            gath_d = dram.tile([N, PAY], bf16)
            nc.sync.dma_start(pay_d.rearrange("(qc p) c -> p qc c", p=P),
                              pay_sb[:])
            nc.gpsimd.collective_compute(
                "AllGather", ALU.bypass,
                replica_groups=[list(range(NCORES))],
                ins=[pay_d.opt()], outs=[gath_d.opt()])
            nc.sync.dma_start(h2g_sb[:],
                              gath_d.rearrange("(b p) c -> p b c", p=P))

            # ---- phase G: layer-2 attention ----
            with tc.tile_pool(name="acc2_psum", bufs=1, space="PSUM") as acc2p, \
                 tc.tile_pool(name="p2_pool", bufs=4) as pp2, \
                 tc.tile_pool(name="small2", bufs=4) as sp2:
                accs2 = []
                for qc in range(QC):
                    accs2.append(acc2p.tile([P, C + 1], f32, tag=f"a2_{qc}", name=f"a2_{qc}"))
                for mb in range(MB):
                    bcol = sp2.tile([P, 1], f32, tag="bcol")
                    nc.vector.tensor_copy(bcol[:], h2g_sb[:, mb, C + 1:C + 2])
                    pm2 = leaky_exp(pp2, e2b[:], bcol[:],
                                    maskT_sb[:, mb, :], mb)
                    for qc in range(QC):
                        nc.tensor.matmul(accs2[qc][:], pm2[:, bass.ts(qc, P)],
                                         h2g_sb[:, mb, 0:C + 1],
                                         start=(mb == 0), stop=(mb == MB - 1))
                for qc in range(QC):
                    r2 = sp2.tile([P, 1], f32, tag="r2")
                    nc.vector.reciprocal(r2[:], accs2[qc][:, C:C + 1])
                    logits = sp2.tile([P, C], f32, tag="logits")
                    nc.vector.tensor_scalar_mul(logits[:], accs2[qc][:, 0:C],
                                                r2[:])
                    negmax = sp2.tile([P, 1], f32, tag="negmax")
                    nc.vector.reduce_max(negmax[:], logits[:], axis=AX.X,
                                         negate=True)
                    expt = sp2.tile([P, C], f32, tag="expt")
                    ssum = sp2.tile([P, 1], f32, tag="ssum")
                    nc.scalar.activation(expt[:], logits[:], AF.Exp,
                                         bias=negmax[:], accum_out=ssum[:])
                    lse = sp2.tile([P, 1], f32, tag="lse")
                    nc.scalar.activation(lse[:], ssum[:], AF.Ln)
                    res = sp2.tile([P, C], f32, tag="res")
                    nc.vector.tensor_scalar(res[:], logits[:], negmax[:],
                                            lse[:], ALU.add, ALU.subtract)
                    nc.sync.dma_start(out_d[bass.ts(qc, P), :], res[:])


_CACHED = None


def _get_nc():
    global _CACHED
    if _CACHED is None:
        _CACHED = _build()
    return _CACHED


def kernel(x, adj, W1, a1, W2, a2):
    x = np.asarray(x, dtype=np.float32)
    adj = np.asarray(adj)
    W1 = np.asarray(W1, dtype=np.float32)
    a1 = np.asarray(a1, dtype=np.float32)
    W2 = np.asarray(W2, dtype=np.float32)
    a2 = np.asarray(a2, dtype=np.float32)

    bf = ml_dtypes.bfloat16
    xT = np.ascontiguousarray(x.T).astype(bf)                     # [F, N]
    # fused score weights: e_src = x @ (W1 @ a_src), e_dst likewise
    wsrc = np.einsum("hfo,ho->fh", W1, a1[:, :O])                 # [F, H]
    wdst = np.einsum("hfo,ho->fh", W1, a1[:, O:])                 # [F, H]
    wsd = np.concatenate([wsrc, wdst], axis=1).astype(bf)         # [F, 2H]
    w1cat = np.concatenate([W1[h] for h in range(H)], 1).astype(bf)  # [F, H*O]
    w2p = np.zeros((O, PAY), np.float32)
    w2p[:, 0:C] = W2[0]
    w2p[:, C] = W2[0] @ a2[0, C:]      # e2_dst vector
    w2p[:, C + 1] = W2[0] @ a2[0, :C]  # e2_src vector
    w2p = w2p.astype(bf)

    adj_on = adj > 0
    in_maps = []
    for c in range(NCORES):
        rows = slice(c * Q, (c + 1) * Q)
        in_maps.append({
            "xT": xT,
            "xTq": np.ascontiguousarray(xT[:, rows]),
            "maskT": np.ascontiguousarray(adj_on[rows, :].T).astype(bf),
            "w1": w1cat,
            "wsd": wsd,
            "w2p": w2p,
        })

    nc = _get_nc()
    res = run_bass_kernel_spmd(nc, in_maps, core_ids=list(range(NCORES)))
    return np.concatenate([res.results[c]["out"] for c in range(NCORES)], 0)


# revision 16
# speedup vs baseline: 1.3485x; 1.0459x over previous
"""2-layer GAT (graph attention) on 8 Trainium2 NeuronCores.

Sharding: query/node rows split 8 ways (512 rows per core). Attention scores are
computed transposed ([key_block=128, q=512]) so the probability tiles feed the
TensorE directly as lhsT with no on-chip transpose of the attention matrix.
Score vectors come from fused weights (e_src = x @ (W1 @ a_src)), leaky-relu and
the bias add ride the ScalarE activation (Prelu with per-partition bias), the
edge mask is applied multiplicatively after exp (identical to -inf masking), and
softmax denominators come free via a ones-column appended to the value matrix.
Layer 2 needs full layer-1 node features; each core computes its 512 rows and a
small packed [512, 18] payload (h2 | ones | e2_dst) is AllGathered.
"""

import numpy as np
import ml_dtypes

import concourse.bass as bass
import concourse.tile as tile
from concourse import bacc, mybir
from concourse.bass_utils import run_bass_kernel_spmd
from concourse.masks import make_identity

P = 128
N, F, O, H, C = 4096, 512, 256, 4, 16
NCORES = 8
Q = N // NCORES          # 512 query rows per core
QC = Q // P              # 4 query chunks
MB = N // P              # 32 key blocks
KB = F // P              # 4 contraction blocks over F
OB = O // P              # 2 contraction blocks over O
ALPHA = 0.2
PAY = C + 2              # payload cols: 0:16 h2, 16 ones, 17 e2_dst
# leaky-relu engine routing, out of every 8 score tiles:
# first ACT_FRAC on ScalarE (Prelu), next DVE_FRAC on VectorE, rest on GpSimd
ACT_FRAC = 4
DVE_FRAC = 1

bf16 = mybir.dt.bfloat16
f32 = mybir.dt.float32
AF = mybir.ActivationFunctionType
ALU = mybir.AluOpType
AX = mybir.AxisListType


def _build(reps=1):
    nc = bacc.Bacc("TRN2", target_bir_lowering=False, debug=False,
                   num_devices=NCORES)

    xTq_d = nc.dram_tensor("xTq", [F, Q], bf16, kind="ExternalInput").ap()
    maskT_d = nc.dram_tensor("maskT", [N, Q], bf16, kind="ExternalInput").ap()
    w1_d = nc.dram_tensor("w1", [F, H * O], bf16, kind="ExternalInput").ap()
    wsd_d = nc.dram_tensor("wsd", [F, 2 * H], bf16, kind="ExternalInput").ap()
    w2p_d = nc.dram_tensor("w2p", [O, PAY], bf16, kind="ExternalInput").ap()
    out_d = nc.dram_tensor("out", [Q, C], f32, kind="ExternalOutput").ap()

    with tile.TileContext(nc) as tc:
        for _ in range(reps):
            _emit(tc, xTq_d, maskT_d, w1_d, wsd_d, w2p_d, out_d)
    nc.compile()
    return nc


def _emit(tc, xTq_d, maskT_d, w1_d, wsd_d, w2p_d, out_d):
    nc = tc.nc
    with tc.tile_pool(name="singles", bufs=1) as singles:
        # ---- persistent SBUF tensors ----
        xTq_sb = singles.tile([P, KB, Q], bf16)
        maskT_sb = singles.tile([P, MB, Q], bf16)
        w1_sb = singles.tile([P, KB, H * O], bf16)
        wsd_sb = singles.tile([P, KB, 2 * H], bf16)
        w2p_sb = singles.tile([P, OB, PAY], bf16)
        ones1 = singles.tile([1, P], f32)
        ident = singles.tile([P, P], bf16)
        HC = H * (O + 1) + 2 * H      # gathered row: 4x(h|1) then 8 e-vals
        h_sb = singles.tile([P, MB, HC], bf16)
        hpay_sb = singles.tile([P, QC, HC], bf16)
        edst_sb = singles.tile([P, MB, H], f32)
        esrcb = singles.tile([P, H, Q], f32)
        x2acc = singles.tile([P, QC, O], f32)
        x2bf = singles.tile([P, QC, O], bf16)
        x2T = singles.tile([P, OB, Q], bf16)
        pay_sb = singles.tile([P, QC, PAY], bf16)
        h2g_sb = singles.tile([P, MB, PAY], bf16)
        e2b = singles.tile([P, Q], f32)
        esrcb_bf = singles.tile([P, H, Q], bf16)
        e2b_bf = singles.tile([P, Q], bf16)

        # ---- input DMAs (split for queue parallelism) ----
        nc.sync.dma_start(xTq_sb[:], xTq_d.rearrange("(kb p) q -> p kb q", p=P))
        maskT_r = maskT_d.rearrange("(b p) q -> p b q", p=P)
        for g in range(8):
            s = bass.ts(g, MB // 8)
            nc.sync.dma_start(maskT_sb[:, s, :], maskT_r[:, s, :])
        w1_r = w1_d.rearrange("(kb p) c -> p kb c", p=P)
        for half in range(2):
            s = bass.ds(half * (H * O // 2), H * O // 2)
            nc.sync.dma_start(w1_sb[:, :, s], w1_r[:, :, s])
        nc.sync.dma_start(wsd_sb[:], wsd_d.rearrange("(kb p) c -> p kb c", p=P))
        nc.sync.dma_start(w2p_sb[:], w2p_d.rearrange("(ob p) c -> p ob c", p=P))

        nc.vector.memset(ones1[:], 1.0)
        make_identity(nc, ident[:])

        # ---- phase B: score vectors ----
        with tc.tile_pool(name="vec_psum", bufs=2, space="PSUM") as vpp, \
             tc.tile_pool(name="vec_sb", bufs=2) as vsb:
            for h in range(H):
                ps_es = vpp.tile([1, Q], f32, tag="ps_es")
                for kb in range(KB):
                    nc.tensor.matmul(ps_es[:], wsd_sb[:, kb, h:h + 1],
                                     xTq_sb[:, kb, :],
                                     start=(kb == 0), stop=(kb == KB - 1))
                esrcT = vsb.tile([1, Q], f32, tag="esrcT")
                nc.vector.tensor_copy(esrcT[:], ps_es[:])
                ps_b = vpp.tile([P, Q], f32, tag="ps_b")
                nc.tensor.matmul(ps_b[:], ones1[:], esrcT[:],
                                 start=True, stop=True)
                nc.vector.tensor_copy(esrcb[:, h, :], ps_b[:])
            nc.vector.tensor_copy(esrcb_bf[:], esrcb[:])

        # ---- phase C: h/e for own rows, AllGather packed payload ----
        with tc.tile_pool(name="h_psum", bufs=2, space="PSUM") as hpp, \
             tc.tile_pool(name="h_dram", bufs=1, space="DRAM") as hdram:
            for h in range(H):
                nc.vector.memset(hpay_sb[:, :, h * (O + 1) + O], 1.0)
            for qc in range(QC):
                ps_h = hpp.tile([P, H * O], f32, tag="ps_h")
                for half in range(2):
                    s = bass.ds(half * 512, 512)
                    for kb in range(KB):
                        nc.tensor.matmul(ps_h[:, s],
                                         xTq_sb[:, kb, bass.ts(qc, P)],
                                         w1_sb[:, kb, s],
                                         start=(kb == 0), stop=(kb == KB - 1))
                for h in range(H):
                    nc.vector.tensor_copy(
                        hpay_sb[:, qc, h * (O + 1):h * (O + 1) + O],
                        ps_h[:, bass.ts(h, O)])
                ps_E = hpp.tile([P, 2 * H], f32, tag="ps_E")
                for kb in range(KB):
                    nc.tensor.matmul(ps_E[:], xTq_sb[:, kb, bass.ts(qc, P)],
                                     wsd_sb[:, kb, :],
                                     start=(kb == 0), stop=(kb == KB - 1))
                nc.vector.tensor_copy(
                    hpay_sb[:, qc, H * (O + 1):], ps_E[:])
            hpay_d = hdram.tile([Q, HC], bf16)
            hgath_d = hdram.tile([N, HC], bf16)
            nc.sync.dma_start(hpay_d.rearrange("(qc p) c -> p qc c", p=P),
                              hpay_sb[:])
            nc.gpsimd.collective_compute(
                "AllGather", ALU.bypass,
                replica_groups=[list(range(NCORES))],
                ins=[hpay_d.opt()], outs=[hgath_d.opt()])
            hg_r = hgath_d.rearrange("(b p) c -> p b c", p=P)
            for g in range(8):
                s = bass.ts(g, MB // 8)
                nc.sync.dma_start(h_sb[:, s, :], hg_r[:, s, :])
            # e_dst back to f32 for use as per-partition activation bias
            nc.vector.tensor_copy(
                edst_sb[:], h_sb[:, :, H * (O + 1) + H:H * (O + 1) + 2 * H])

        # ---- phase D: layer-1 attention, head-outer ----
        def leaky_exp(pool, esrc_ap, bias_ap, mask_ap, idx, esrc_bf_ap=None,
                      bias_bf_ap=None):
            """p = exp(leaky(esrc + bias)) * mask, with the leaky pass routed
            across engines by tile index to balance the attention phase.
            The DVE route runs in bf16 (2x mode) when bf16 operands given."""
            route = idx % 8
            if route < ACT_FRAC or esrc_bf_ap is None:
                zt = pool.tile([P, Q], f32, tag="zt", name="zt")
                nc.scalar.activation(zt[:], esrc_ap, AF.Prelu, bias=bias_ap,
                                     scale=1.0, alpha=ALPHA)
            else:
                z0 = pool.tile([P, Q], bf16, tag="z0", name="z0")
                nc.vector.tensor_scalar_add(z0[:], esrc_bf_ap, bias_ap)
                zt = pool.tile([P, Q], bf16, tag="zt_b", name="zt_b")
                nc.vector.scalar_tensor_tensor(zt[:], z0[:], ALPHA, z0[:],
                                               op0=ALU.mult, op1=ALU.max)
            pt = pool.tile([P, Q], bf16, tag="pt", name="pt")
            nc.scalar.activation(pt[:], zt[:], AF.Exp)
            pm = pool.tile([P, Q], bf16, tag="pm", name="pm")
            nc.vector.tensor_mul(pm[:], pt[:], mask_ap)
            return pm

        with tc.tile_pool(name="acc_psum", bufs=1, space="PSUM") as accp, \
             tc.tile_pool(name="p_pool", bufs=4) as pp, \
             tc.tile_pool(name="small1", bufs=4) as sp1:
            for h in range(H):
                accs = []
                for qc in range(QC):
                    accs.append(accp.tile([P, O + 1], f32, tag=f"acc{qc}", name=f"acc{qc}"))
                for mb in range(MB):
                    pm = leaky_exp(pp, esrcb[:, h, :],
                                   edst_sb[:, mb, h:h + 1],
                                   maskT_sb[:, mb, :], h * MB + mb,
                                   esrcb_bf[:, h, :])
                    for qc in range(QC):
                        nc.tensor.matmul(
                            accs[qc][:], pm[:, bass.ts(qc, P)],
                            h_sb[:, mb, h * (O + 1):(h + 1) * (O + 1)],
                            start=(mb == 0), stop=(mb == MB - 1))
                for qc in range(QC):
                    r = sp1.tile([P, 1], f32, tag="r")
                    nc.vector.reciprocal(r[:], accs[qc][:, O:O + 1])
                    nc.vector.tensor_scalar_mul(r[:], r[:], 1.0 / H)
                    if h == 0:
                        nc.vector.tensor_scalar_mul(x2acc[:, qc, :],
                                                    accs[qc][:, 0:O], r[:])
                    else:
                        nc.vector.scalar_tensor_tensor(
                            x2acc[:, qc, :], accs[qc][:, 0:O], r[:],
                            x2acc[:, qc, :], op0=ALU.mult, op1=ALU.add)

        # ---- phase E: relu, transpose, layer-2 projections ----
        nc.vector.tensor_relu(x2bf[:], x2acc[:])
        with tc.tile_pool(name="l2_psum", bufs=2, space="PSUM") as lpp, \
             tc.tile_pool(name="l2_sb", bufs=2) as lsb:
            for qc in range(QC):
                for ob in range(OB):
                    tp = lpp.tile([P, P], bf16, tag="tp")
                    nc.tensor.transpose(tp[:], x2bf[:, qc, bass.ts(ob, P)],
                                        ident[:])
                    nc.vector.tensor_copy(x2T[:, ob, bass.ts(qc, P)], tp[:])
            for qc in range(QC):
                ps2 = lpp.tile([P, C + 1], f32, tag="ps2")
                for ob in range(OB):
                    nc.tensor.matmul(ps2[:], x2T[:, ob, bass.ts(qc, P)],
                                     w2p_sb[:, ob, 0:C + 1],
                                     start=(ob == 0), stop=(ob == OB - 1))
                nc.vector.tensor_copy(pay_sb[:, qc, 0:C], ps2[:, 0:C])
                nc.vector.tensor_copy(pay_sb[:, qc, C + 1:C + 2],
                                      ps2[:, C:C + 1])
            nc.vector.memset(pay_sb[:, :, C], 1.0)
            ps_e2 = lpp.tile([1, Q], f32, tag="ps_e2")
            for ob in range(OB):
                nc.tensor.matmul(ps_e2[:], w2p_sb[:, ob, C + 1:C + 2],
                                 x2T[:, ob, :],
                                 start=(ob == 0), stop=(ob == OB - 1))
            e2srcT = lsb.tile([1, Q], f32, tag="e2srcT")
            nc.vector.tensor_copy(e2srcT[:], ps_e2[:])
            ps_b2 = lpp.tile([P, Q], f32, tag="ps_b2")
            nc.tensor.matmul(ps_b2[:], ones1[:], e2srcT[:],
                             start=True, stop=True)
            nc.vector.tensor_copy(e2b[:], ps_b2[:])
            nc.vector.tensor_copy(e2b_bf[:], ps_b2[:])

        # ---- phase F: AllGather packed payload ----
        with tc.tile_pool(name="dram", bufs=1, space="DRAM") as dram:
            pay_d = dram.tile([Q, PAY], bf16)
            gath_d = dram.tile([N, PAY], bf16)
            nc.sync.dma_start(pay_d.rearrange("(qc p) c -> p qc c", p=P),
                              pay_sb[:])
            nc.gpsimd.collective_compute(
                "AllGather", ALU.bypass,
                replica_groups=[list(range(NCORES))],
                ins=[pay_d.opt()], outs=[gath_d.opt()])
            nc.sync.dma_start(h2g_sb[:],
                              gath_d.rearrange("(b p) c -> p b c", p=P))

            # ---- phase G: layer-2 attention ----
            with tc.tile_pool(name="acc2_psum", bufs=1, space="PSUM") as acc2p, \
                 tc.tile_pool(name="p2_pool", bufs=4) as pp2, \
                 tc.tile_pool(name="small2", bufs=4) as sp2:
                accs2 = []
                for qc in range(QC):
                    accs2.append(acc2p.tile([P, C + 1], f32, tag=f"a2_{qc}", name=f"a2_{qc}"))
                for mb in range(MB):
                    bcol = sp2.tile([P, 1], f32, tag="bcol")
                    nc.vector.tensor_copy(bcol[:], h2g_sb[:, mb, C + 1:C + 2])
                    pm2 = leaky_exp(pp2, e2b[:], bcol[:],
                                    maskT_sb[:, mb, :], mb, e2b_bf[:])
                    for qc in range(QC):
                        nc.tensor.matmul(accs2[qc][:], pm2[:, bass.ts(qc, P)],
                                         h2g_sb[:, mb, 0:C + 1],
                                         start=(mb == 0), stop=(mb == MB - 1))
                for qc in range(QC):
                    r2 = sp2.tile([P, 1], f32, tag="r2")
                    nc.vector.reciprocal(r2[:], accs2[qc][:, C:C + 1])
                    logits = sp2.tile([P, C], f32, tag="logits")
                    nc.vector.tensor_scalar_mul(logits[:], accs2[qc][:, 0:C],
                                                r2[:])
                    negmax = sp2.tile([P, 1], f32, tag="negmax")
                    nc.vector.reduce_max(negmax[:], logits[:], axis=AX.X,
                                         negate=True)
                    expt = sp2.tile([P, C], f32, tag="expt")
                    ssum = sp2.tile([P, 1], f32, tag="ssum")
                    nc.scalar.activation(expt[:], logits[:], AF.Exp,
                                         bias=negmax[:], accum_out=ssum[:])
                    lse = sp2.tile([P, 1], f32, tag="lse")
                    nc.scalar.activation(lse[:], ssum[:], AF.Ln)
                    res = sp2.tile([P, C], f32, tag="res")
                    nc.vector.tensor_scalar(res[:], logits[:], negmax[:],
                                            lse[:], ALU.add, ALU.subtract)
                    nc.sync.dma_start(out_d[bass.ts(qc, P), :], res[:])


_CACHED = None


def _get_nc():
    global _CACHED
    if _CACHED is None:
        _CACHED = _build()
    return _CACHED


def kernel(x, adj, W1, a1, W2, a2):
    x = np.asarray(x, dtype=np.float32)
    adj = np.asarray(adj)
    W1 = np.asarray(W1, dtype=np.float32)
    a1 = np.asarray(a1, dtype=np.float32)
    W2 = np.asarray(W2, dtype=np.float32)
    a2 = np.asarray(a2, dtype=np.float32)

    bf = ml_dtypes.bfloat16
    xT = np.ascontiguousarray(x.T).astype(bf)                     # [F, N]
    # fused score weights: e_src = x @ (W1 @ a_src), e_dst likewise
    wsrc = np.einsum("hfo,ho->fh", W1, a1[:, :O])                 # [F, H]
    wdst = np.einsum("hfo,ho->fh", W1, a1[:, O:])                 # [F, H]
    wsd = np.concatenate([wsrc, wdst], axis=1).astype(bf)         # [F, 2H]
    w1cat = np.concatenate([W1[h] for h in range(H)], 1).astype(bf)  # [F, H*O]
    w2p = np.zeros((O, PAY), np.float32)
    w2p[:, 0:C] = W2[0]
    w2p[:, C] = W2[0] @ a2[0, C:]      # e2_dst vector
    w2p[:, C + 1] = W2[0] @ a2[0, :C]  # e2_src vector
    w2p = w2p.astype(bf)

    adj_on = adj > 0
    in_maps = []
    for c in range(NCORES):
        rows = slice(c * Q, (c + 1) * Q)
        in_maps.append({
            "xTq": np.ascontiguousarray(xT[:, rows]),
            "maskT": np.ascontiguousarray(adj_on[rows, :].T).astype(bf),
            "w1": w1cat,
            "wsd": wsd,
            "w2p": w2p,
        })

    nc = _get_nc()
    res = run_bass_kernel_spmd(nc, in_maps, core_ids=list(range(NCORES)))
    return np.concatenate([res.results[c]["out"] for c in range(NCORES)], 0)
